# revision 29
# baseline (speedup 1.0000x reference)
"""Trainium2 Bass kernel for nn_Decoder (4-layer dense transformer decoder).

Sharding (8 NeuronCores):
  - Sequence-parallel residual stream: core c owns tokens [256c, 256c+256).
  - qkv is computed by each token's owner for ALL heads (RoPE applied
    locally, position known), then one 1.5MB AllToAll delivers each core the
    full-sequence q/k/v for its 2 heads.  This replaces a 4MB AllGather of
    activations (120us -> 54us per layer).
  - Attention outputs return to token-local form via a small AllToAll (0.5MB).
  - The softmax denominator is folded into the AV matmul via a ones-column
    appended to V (no separate PE sum matmuls).
  - FFN / out-proj / logits weights are replicated (bf16) and streamed from
    HBM in >=512B contiguous runs (host pre-transposes), overlapped with
    compute; gamma, sqrt(D) and the attention 1/sqrt(dh) scale are folded
    into weights on the host; biases ride the ACT activation ops.

Layout: everything on-chip is transposed - [D(partitions), tokens(free)] -
so RMS-norm scaling, gelu bias and per-channel ops are native per-partition
ops, and matmul lhsT slices come straight from the weight matrices.

PSUM budget (8 banks of 2KB):
  acc_a(1) acc_b(1) big_a(2) big_b(2): attention pav/pb + score-pair
  buffers; FFN reuses big_a/big_b as the 8-chunk ff2 accumulators and the
  logits loop uses all four as a 4-deep pipeline.  "ps" tag (2 bufs x 2KB)
  carries all short-lived single matmul outputs.
"""
import numpy as np
import ml_dtypes

import concourse.bass as bass
import concourse.mybir as mybir
import concourse.tile as tile
from concourse import bacc
from concourse.tile import TileContext
from concourse.masks import make_identity

BF16 = np.dtype(ml_dtypes.bfloat16)
AF = mybir.ActivationFunctionType
P = 128

# model dims
V, D, DEPTH, H, DH, FF = 32000, 1024, 4, 16, 64, 4096
B, N = 1, 2048
NC = 8  # cores


class Cfg:
    def __init__(self, n=N, depth=DEPTH, v=V, ff=FF):
        self.n = n            # total tokens
        self.depth = depth
        self.v = v
        self.ff = ff
        self.t = n // NC      # tokens per core
        self.tb = self.t // P           # token blocks per core (2)
        self.db = D // P                # D blocks (8)
        self.fb = ff // P               # FF blocks (32)
        self.heads_per_core = H // NC   # 2
        self.hd = self.heads_per_core * DH  # 128 head-dims per core
        self.n_kb = n // P              # key blocks (16)
        self.qc_w = min(512, n)         # query chunk width
        self.n_qc = n // self.qc_w      # query chunks
        self.kb_per_qc = self.qc_w // P  # 4


FULL = Cfg()

# keep-warm train lengths (junk matmuls of 512 rows each, ~213ns apiece)
import os as _os
WARM_T1 = int(_os.environ.get("WARM_T1", 40))
WARM_T2 = int(_os.environ.get("WARM_T2", 0))
WARM_T3 = int(_os.environ.get("WARM_T3", 0))
WARM_T4 = int(_os.environ.get("WARM_T4", 0))
Q_ATT_STAGE = _os.environ.get("Q_ATT_STAGE", "act")
Q_ATTC = _os.environ.get("Q_ATTC", "sp")


def build_kernel(cfg=FULL):
    n, t, depth, v, ff = cfg.n, cfg.t, cfg.depth, cfg.v, cfg.ff
    db, fb, tb = cfg.db, cfg.fb, cfg.tb
    hd = cfg.hd
    hpc = cfg.heads_per_core
    f32, bf16, i32 = mybir.dt.float32, mybir.dt.bfloat16, mybir.dt.int32

    nc = bacc.Bacc(None, target_bir_lowering=False, debug=False, num_devices=NC)

    # ---------- DRAM I/O ----------
    embT = nc.dram_tensor("embT", [P, db, t], f32, kind="ExternalInput")
    # full qkv weights: [depth, 3, D, H*DH] (q pre-scaled, gamma folded)
    wqkv = nc.dram_tensor("wqkv", [depth, 3, D, D], bf16, kind="ExternalInput")
    # out-proj weights, host-transposed to [depth, db, P, NC, P]
    wout = nc.dram_tensor("wout", [depth, db, P, NC, P], bf16,
                          kind="ExternalInput")
    # ff1 weights, host-transposed to [depth, fb, P, db, P]
    wff1 = nc.dram_tensor("wff1", [depth, fb, P, db, P], bf16,
                          kind="ExternalInput")
    bff1 = nc.dram_tensor("bff1", [depth, P, fb], f32, kind="ExternalInput")
    wff2 = nc.dram_tensor("wff2", [depth, ff, D], bf16, kind="ExternalInput")
    bff2 = nc.dram_tensor("bff2", [depth, P, db], f32, kind="ExternalInput")
    wlog = nc.dram_tensor("wlog", [D, v], bf16, kind="ExternalInput")
    # local-position rope tables [P, t] (2-head tile pattern, per-core slice)
    cosq = nc.dram_tensor("cosq", [P, t], bf16, kind="ExternalInput")
    sinq = nc.dram_tensor("sinq", [P, t], bf16, kind="ExternalInput")
    rotPT = nc.dram_tensor("rotPT", [P, P], bf16, kind="ExternalInput")
    # diagonal causal masks for the 512-wide q-chunk: j = kb offset in chunk
    trimask = nc.dram_tensor("trimask", [cfg.kb_per_qc, P, cfg.qc_w], bf16,
                             kind="ExternalInput")
    logits_out = nc.dram_tensor("logits_out", [t, v], bf16,
                                kind="ExternalOutput")

    # collective bounce buffers (reused across layers)
    # qkv scatter: chunk g = (q[P,t], k[P,t], vT[t,P]) for head-group g
    qkv_in = [nc.dram_tensor(f"qkv_in{i}", [NC, 3, DH * t], bf16)
              for i in range(hpc)]
    qkv_out = [nc.dram_tensor(f"qkv_out{i}", [NC, 3, DH * t], bf16)
               for i in range(hpc)]
    a2a_in = [nc.dram_tensor(f"a2a_in{i}", [NC, DH, t], bf16)
              for i in range(hpc)]
    a2a_out = [nc.dram_tensor(f"a2a_out{i}", [NC, DH, t], bf16)
               for i in range(hpc)]

    with TileContext(nc) as tc:
        with tc.tile_pool(name="const", bufs=1) as cpool, \
             tc.tile_pool(name="resid", bufs=1) as rpool, \
             tc.tile_pool(name="work", bufs=1) as wpool, \
             tc.tile_pool(name="qkvw", bufs=1) as qwpool, \
             tc.tile_pool(name="wts", bufs=3) as wtpool, \
             tc.tile_pool(name="small", bufs=3) as spool, \
             tc.tile_pool(name="attn", bufs=1) as apool, \
             tc.tile_pool(name="pexp", bufs=3) as epool, \
             tc.tile_pool(name="psum_acc", bufs=1, space="PSUM") as pacc, \
             tc.tile_pool(name="psum_s", bufs=2, space="PSUM") as ps:

            # ---------- constants ----------
            ones_bf = cpool.tile([P, 1], bf16)
            nc.vector.memset(ones_bf[:], 1.0)
            ones_row = cpool.tile([1, P], f32)
            nc.vector.memset(ones_row[:], 1.0)
            cos_t = cpool.tile([P, t], bf16)
            sin_t = cpool.tile([P, t], bf16)
            rot_t = cpool.tile([P, P], bf16)
            nc.sync.dma_start(cos_t[:], cosq[:, :])
            nc.sync.dma_start(sin_t[:], sinq[:, :])
            nc.sync.dma_start(rot_t[:], rotPT[:, :])
            mask_t = cpool.tile([P, cfg.kb_per_qc, cfg.qc_w], bf16)
            nc.sync.dma_start(
                mask_t[:], trimask[:, :, :].rearrange("j p q -> p j q"))

            def warm_pe(nmm, name):
                """Keep the tensor engine p-state ramped through a known
                idle window: a train of dependency-free junk matmuls on
                constants, accumulating into one scratch psum bank."""
                if nmm <= 0:
                    return
                wtile = ps.tile([P, cfg.qc_w], f32, tag="ps", name=name)
                for i in range(nmm):
                    nc.tensor.matmul(wtile[:], rot_t[:], mask_t[:, 0, :],
                                     start=(i == 0), stop=(i == nmm - 1))

            # persistent attention tiles (written every layer)
            vt = apool.tile([P, cfg.n_kb, hpc, DH + 1], bf16)
            nc.vector.memset(vt[:, :, :, DH:], 1.0)
            # pre-zero the score psum banks: narrowed diagonal QK matmuls
            # leave stale bank regions that flow through exp (masked to zero
            # later) -- stale NaNs on first use would poison that path
            z_a = pacc.tile([P, 2, cfg.qc_w], f32, tag="big_a", name="z_a")
            nc.vector.memset(z_a[:], 0.0)
            z_b = pacc.tile([P, 2, cfg.qc_w], f32, tag="big_b", name="z_b")
            nc.vector.memset(z_b[:], 0.0)

            # ---------- embedding rows (host-gathered) -> hT [P, db, t] ----
            hT = rpool.tile([P, db, t], f32)
            nc.sync.dma_start(hT[:], embT[:, :, :])
            warm_pe(WARM_T1, "warm_t1")

            def rms_norm_cast(src_f32, dst_bf):
                """dst_bf[P, db, t] = src * rsqrt(sum_D(src^2)); sqrt(D)*gamma
                is folded into the consuming weights."""
                sq = wpool.tile([P, db, t], bf16, tag="normsq")
                for dbi in range(db):
                    nc.vector.tensor_tensor(
                        sq[:, dbi, :], src_f32[:, dbi, :], src_f32[:, dbi, :],
                        mybir.AluOpType.mult)
                psum_n = pacc.tile([1, t], f32, tag="acc_a", name="psum_n")
                for dbi in range(db):
                    nc.tensor.matmul(psum_n[:], ones_bf[:], sq[:, dbi, :],
                                     start=(dbi == 0), stop=(dbi == db - 1))
                rt = spool.tile([1, t], f32, tag="norm_rt")
                nc.scalar.activation(rt[:], psum_n[:], AF.Sqrt)
                inv = spool.tile([1, t], f32, tag="norm_inv")
                nc.vector.reciprocal(inv[:], rt[:])
                psum_b = ps.tile([P, t], f32, tag="ps", name="psum_b")
                nc.tensor.matmul(psum_b[:], ones_row[:], inv[:],
                                 start=True, stop=True)
                for dbi in range(db):
                    nc.vector.tensor_tensor(
                        dst_bf[:, dbi, :], src_f32[:, dbi, :], psum_b[:],
                        mybir.AluOpType.mult)

            # ================= layers =================
            for l in range(depth):
                # ----- norm1 -----
                xn = wpool.tile([P, db, t], bf16, tag="xn")
                rms_norm_cast(hT, xn)

                # ----- qkv for ALL heads of local tokens + rope, scatter ---
                wq_t = qwpool.tile([P, db, D], bf16, tag="wq")
                nc.sync.dma_start(
                    wq_t[:], wqkv[l, 0, :, :].rearrange("(o p) c -> p o c",
                                                        p=P))
                wk_t = qwpool.tile([P, db, D], bf16, tag="wk")
                nc.sync.dma_start(
                    wk_t[:], wqkv[l, 1, :, :].rearrange("(o p) c -> p o c",
                                                        p=P))
                wv_t = qwpool.tile([P, db, D], bf16, tag="wv")
                nc.sync.dma_start(
                    wv_t[:], wqkv[l, 2, :, :].rearrange("(o p) c -> p o c",
                                                        p=P))
                # send tiles: q/k [P(ch), NC(group), t]; v [P(tok), NC, tb, P]
                qs = apool.tile([P, NC, t], bf16, tag="qsend")
                ks = apool.tile([P, NC, t], bf16, tag="ksend")
                vs = apool.tile([P, NC, tb, P], bf16, tag="vsend")
                for g in range(NC):
                    gsl = slice(g * P, (g + 1) * P)
                    for wt, dst in ((wq_t, qs), (wk_t, ks)):
                        pq = ps.tile([P, t], f32, tag="ps", name="pq")
                        for dbi in range(db):
                            nc.tensor.matmul(
                                pq[:], wt[:, dbi, gsl], xn[:, dbi, :],
                                start=(dbi == 0), stop=(dbi == db - 1))
                        raw = epool.tile([P, t], bf16, tag="rope_raw")
                        nc.vector.tensor_copy(raw[:], pq[:])
                        prot = ps.tile([P, t], f32, tag="ps", name="prot")
                        nc.tensor.matmul(prot[:], rot_t[:], raw[:],
                                         start=True, stop=True)
                        t1 = epool.tile([P, t], bf16, tag="rope_t1")
                        nc.vector.tensor_tensor(
                            t1[:], prot[:], sin_t[:], mybir.AluOpType.mult)
                        t2 = epool.tile([P, t], bf16, tag="rope_t2")
                        nc.vector.tensor_tensor(
                            t2[:], raw[:], cos_t[:], mybir.AluOpType.mult)
                        nc.vector.tensor_tensor(
                            dst[:, g, :], t1[:], t2[:], mybir.AluOpType.add)
                    # vT chunks: [tokens, ch] via swapped operands
                    for tbi in range(tb):
                        pv = ps.tile([P, P], f32, tag="ps", name="pv")
                        for dbi in range(db):
                            nc.tensor.matmul(
                                pv[:], xn[:, dbi, tbi * P:(tbi + 1) * P],
                                wv_t[:, dbi, gsl],
                                start=(dbi == 0), stop=(dbi == db - 1))
                        nc.vector.tensor_copy(vs[:, g, tbi, :], pv[:])
                    nc.sync.dma_start(
                        qkv_in[0][g, 0].rearrange("(p t) -> p t", p=DH),
                        qs[:DH, g, :])
                    nc.sync.dma_start(
                        qkv_in[0][g, 1].rearrange("(p t) -> p t", p=DH),
                        ks[:DH, g, :])
                    nc.sync.dma_start(
                        qkv_in[0][g, 2].rearrange(
                            "(p tb c) -> p tb c", p=P, tb=tb),
                        vs[:, g, :, :DH])
                # half-1 staged in 3 batched DMAs (flows during the first A2A)
                nc.sync.dma_start(
                    qkv_in[1][:, 0].rearrange("g (p t) -> p g t", p=DH),
                    qs[DH:, :, :])
                nc.sync.dma_start(
                    qkv_in[1][:, 1].rearrange("g (p t) -> p g t", p=DH),
                    ks[DH:, :, :])
                for g in range(NC):
                    nc.sync.dma_start(
                        qkv_in[1][g, 2].rearrange(
                            "(p tb c) -> p tb c", p=P, tb=tb),
                        vs[:, g, :, DH:])
                for hh in range(hpc):
                    nc.gpsimd.collective_compute(
                        "AllToAll", mybir.AluOpType.bypass,
                        replica_groups=[list(range(NC))],
                        ins=[qkv_in[hh].ap().opt()],
                        outs=[qkv_out[hh].ap().opt()])

                warm_pe(WARM_T2, f"warm_t2_{l}")

                # ----- receive: qT/kT [P, n]; vt [P(tok), kb, 2, 65] -----
                qT = apool.tile([P, n], bf16, tag="qT")
                kT = apool.tile([P, n], bf16, tag="kT")
                for hh in range(hpc):
                    csl = slice(hh * DH, (hh + 1) * DH)
                    nc.sync.dma_start(
                        qT[csl, :].rearrange("p (s t) -> p s t", s=NC),
                        qkv_out[hh][:, 0].rearrange("s (p t) -> p s t", p=DH))
                    nc.sync.dma_start(
                        kT[csl, :].rearrange("p (s t) -> p s t", s=NC),
                        qkv_out[hh][:, 1].rearrange("s (p t) -> p s t", p=DH))
                    for s in range(NC):
                        nc.sync.dma_start(
                            vt[:, s * tb:(s + 1) * tb, hh, :DH],
                            qkv_out[hh][s, 2].rearrange(
                                "(p tb c) -> p tb c", p=P, tb=tb))

                # prefetch out-proj weights + FFN biases during attention
                wocs = []
                for dci in range(db):
                    woc = wtpool.tile([P, NC, P], bf16, tag="woc", bufs=8)
                    nc.sync.dma_start(woc[:], wout[l, dci])
                    wocs.append(woc)
                b1 = spool.tile([P, fb], f32, tag="b1")
                nc.sync.dma_start(b1[:], bff1[l, :, :])
                b2 = spool.tile([P, db], f32, tag="b2")
                nc.sync.dma_start(b2[:], bff2[l, :, :])

                # ----- attention (2 heads), output attT [P, n] bf16 -----
                attT = apool.tile([P, n], bf16, tag="attT")
                for h in range(hpc):
                    if h == 1:
                        warm_pe(WARM_T3, f"warm_t3_{l}")
                    hsl = slice(h * DH, (h + 1) * DH)
                    for qc_i in range(cfg.n_qc):
                        qsl = slice(qc_i * cfg.qc_w, (qc_i + 1) * cfg.qc_w)
                        n_kb_q = (qc_i + 1) * cfg.kb_per_qc
                        pav = pacc.tile([DH + 1, cfg.qc_w], f32, tag="acc_a",
                                        name="pav")
                        # key blocks in pairs: 2 QK matmuls -> one batched
                        # exp -> (mask) -> 2 AV accumulates
                        for kp in range(n_kb_q // 2):
                            kb0 = 2 * kp
                            pscr = pacc.tile([P, 2, cfg.qc_w], f32,
                                             tag=f"big_{'ab'[kp % 2]}",
                                             name="pscr")
                            jds = [kb0 + j - qc_i * cfg.kb_per_qc
                                   for j in range(2)]
                            # queries before the diagonal block are fully
                            # masked: narrow the matmul spans
                            offs = [max(0, jj) * P if jj > 0 else 0
                                    for jj in jds]
                            for j in range(2):
                                q0 = qc_i * cfg.qc_w + offs[j]
                                nc.tensor.matmul(
                                    pscr[:, j, offs[j]:],
                                    kT[hsl, (kb0 + j) * P:(kb0 + j + 1) * P],
                                    qT[hsl, q0:(qc_i + 1) * cfg.qc_w],
                                    start=True, stop=True)
                            pe = epool.tile([P, 2, cfg.qc_w], bf16,
                                            tag="att_exp")
                            nc.scalar.activation(pe[:], pscr[:], AF.Exp)
                            if jds[0] >= 0:
                                nc.vector.tensor_tensor(
                                    pe[:], pe[:], mask_t[:, jds[0]:jds[0] + 2,
                                                         :],
                                    mybir.AluOpType.mult)
                            for j in range(2):
                                nc.tensor.matmul(
                                    pav[:, offs[j]:],
                                    vt[:, kb0 + j, h, :],
                                    pe[:, j, offs[j]:],
                                    start=(kb0 + j == 0),
                                    stop=(kb0 + j == n_kb_q - 1))
                        inv = spool.tile([1, cfg.qc_w], f32, tag="att_inv")
                        nc.vector.reciprocal(inv[:], pav[DH:, :])
                        pb = pacc.tile([DH, cfg.qc_w], f32, tag="acc_b",
                                       name="pb")
                        nc.tensor.matmul(pb[:], ones_row[:, :DH], inv[:],
                                         start=True, stop=True)
                        invb = spool.tile([DH, cfg.qc_w], f32, tag="att_invb")
                        nc.vector.tensor_copy(invb[:], pb[:])
                        nc.vector.tensor_tensor(
                            attT[hsl, qsl], pav[:DH, :], invb[:],
                            mybir.AluOpType.mult)
                    # scatter this head's rows while the next head computes
                    # (ACT queue: the SP queue is blocked by half-1 readbacks;
                    # high priority so the scheduler doesn't sink the trigger
                    # behind the next head's exps)
                    eng_s = nc.scalar if Q_ATT_STAGE == "act" else nc.sync
                    with tc.high_priority():
                        eng_s.dma_start(
                            a2a_in[h][:, :, :].rearrange("c d t -> d c t"),
                            attT[hsl, :].rearrange("d (c t) -> d c t", c=NC))
                        nc.gpsimd.collective_compute(
                            "AllToAll", mybir.AluOpType.bypass,
                            replica_groups=[list(range(NC))],
                            ins=[a2a_in[h].ap().opt()],
                            outs=[a2a_out[h].ap().opt()])

                attC = wpool.tile([P, NC, t], bf16, tag="attC")
                eng_c = nc.scalar if Q_ATTC == "act" else nc.sync
                with tc.high_priority():
                    for hh in range(hpc):
                        eng_c.dma_start(
                            attC[hh * DH:(hh + 1) * DH, :, :],
                            a2a_out[hh][:, :, :].rearrange("c d t -> d c t"))
                warm_pe(WARM_T4, f"warm_t4_{l}")

                # ----- out-proj + residual -----
                for dci in range(db):
                    po = ps.tile([P, t], f32, tag="ps", name="po")
                    for hb in range(NC):
                        nc.tensor.matmul(po[:], wocs[dci][:, hb, :],
                                         attC[:, hb, :],
                                         start=(hb == 0), stop=(hb == NC - 1))
                    nc.vector.tensor_tensor(hT[:, dci, :], hT[:, dci, :],
                                            po[:], mybir.AluOpType.add)

                # ----- norm2 + FFN (token-local, no collective) -----
                xn2 = wpool.tile([P, db, t], bf16, tag="xn")
                rms_norm_cast(hT, xn2)
                # fused ff1 -> gelu -> ff2: two 2-bank psum accumulators hold
                # the 8 D-chunk outputs; act chunk is transient.
                pg_a = pacc.tile([P, 4, t], f32, tag="big_a",
                                 name=f"ffacc_a{l}")
                pg_b = pacc.tile([P, 4, t], f32, tag="big_b",
                                 name=f"ffacc_b{l}")
                pgs = [pg_a, pg_b]
                for fci in range(fb):
                    w1c = wtpool.tile([P, db, P], bf16, tag="w1c")
                    nc.sync.dma_start(w1c[:], wff1[l, fci])
                    pf = ps.tile([P, t], f32, tag="ps", name="pf")
                    for dbi in range(db):
                        nc.tensor.matmul(
                            pf[:], w1c[:, dbi, :], xn2[:, dbi, :],
                            start=(dbi == 0), stop=(dbi == db - 1))
                    act_c = epool.tile([P, t], bf16, tag="act_c")
                    nc.scalar.activation(act_c[:], pf[:], AF.Gelu,
                                         bias=b1[:, fci:fci + 1])
                    w2c = wtpool.tile([P, db, P], bf16, tag="w2c")
                    nc.sync.dma_start(
                        w2c[:], wff2[l, fci * P:(fci + 1) * P, :].rearrange(
                            "p (dc q) -> p dc q", q=P))
                    for dci in range(db):
                        nc.tensor.matmul(
                            pgs[dci // 4][:, dci % 4, :], w2c[:, dci, :],
                            act_c[:],
                            start=(fci == 0 and dci % 2 == 0),
                            stop=(fci == fb - 1 and dci % 2 == 1))
                for dci in range(db):
                    tmp = spool.tile([P, t], f32, tag="ff2_tmp")
                    nc.scalar.activation(tmp[:], pgs[dci // 4][:, dci % 4, :],
                                         AF.Identity, bias=b2[:, dci:dci + 1])
                    nc.vector.tensor_tensor(hT[:, dci, :], hT[:, dci, :],
                                            tmp[:], mybir.AluOpType.add)

            # ================= final norm + logits =================
            xnf = wpool.tile([P, db, t], bf16, tag="xn")
            rms_norm_cast(hT, xnf)
            vchunks = []
            off = 0
            while off < v:
                w = min(512, v - off)
                vchunks.append((off, w))
                off += w
            ptags = ["acc_a", "acc_b", "big_a", "big_b"]
            for ci, (off, w) in enumerate(vchunks):
                wl = wtpool.tile([P, db, 512], bf16, tag="wl", bufs=4)
                nc.sync.dma_start(
                    wl[:, :, :w],
                    wlog[:, off:off + w].rearrange("(o p) c -> p o c", p=P))
                for tbi in range(tb):
                    pl = pacc.tile([P, 512], f32,
                                   tag=ptags[(2 * ci + tbi) % 4],
                                   name=f"pl_{off}_{tbi}")
                    for dbi in range(db):
                        nc.tensor.matmul(
                            pl[:, :w], xnf[:, dbi, tbi * P:(tbi + 1) * P],
                            wl[:, dbi, :w],
                            start=(dbi == 0), stop=(dbi == db - 1))
                    ot = spool.tile([P, 512], bf16, tag="log_out")
                    nc.vector.tensor_copy(ot[:, :w], pl[:, :w])
                    nc.scalar.dma_start(
                        logits_out[tbi * P:(tbi + 1) * P, off:off + w],
                        ot[:, :w])
    nc.finalize()
    return nc


# ======================= host side =======================

def prep_inputs(inputs, cfg=FULL):
    """Full model inputs -> list of 8 per-core input dicts (numpy)."""
    n, t, depth, v, ff = cfg.n, cfg.t, cfg.depth, cfg.v, cfg.ff
    x = np.asarray(inputs["x"]).reshape(-1)[:n].astype(np.int32)
    emb = np.asarray(inputs["token_emb"], dtype=np.float32)[:v]
    attn_g = np.asarray(inputs["attn_gamma"], dtype=np.float32)
    w_qkv = np.asarray(inputs["w_qkv"], dtype=np.float32)
    w_out = np.asarray(inputs["w_attn_out"], dtype=np.float32)
    ff_g = np.asarray(inputs["ff_gamma"], dtype=np.float32)
    w_ff1 = np.asarray(inputs["w_ff1"], dtype=np.float32)[:, :, :ff]
    b_ff1 = np.asarray(inputs["b_ff1"], dtype=np.float32)[:, :ff]
    w_ff2 = np.asarray(inputs["w_ff2"], dtype=np.float32)[:, :ff, :]
    b_ff2 = np.asarray(inputs["b_ff2"], dtype=np.float32)
    fin_g = np.asarray(inputs["final_gamma"], dtype=np.float32)
    w_log = np.asarray(inputs["w_logits"], dtype=np.float32)[:, :v]

    sD = float(np.sqrt(D))
    # rope tables for LOCAL positions (per-core slice below)
    inv_freq = 1.0 / (10000.0 ** (np.arange(0, DH, 2, dtype=np.float32) / DH))
    freqs = np.arange(n, dtype=np.float32)[:, None] * inv_freq[None, :]
    freqs = np.repeat(freqs, 2, axis=-1)          # [n, DH]
    cos = np.cos(freqs).T                          # [DH, n]
    sin = np.sin(freqs).T
    cos2 = np.tile(cos, (cfg.heads_per_core, 1)).astype(BF16)   # [128, n]
    sin2 = np.tile(sin, (cfg.heads_per_core, 1)).astype(BF16)
    # rot matrix PT st rot(x) = PT.T @ x, block-diag per head (DH x DH)
    Pm = np.zeros((DH, DH), np.float32)
    for i in range(0, DH, 2):
        Pm[i, i + 1] = -1.0
        Pm[i + 1, i] = 1.0
    PT1 = Pm.T
    PT = np.zeros((P, P), np.float32)
    for h in range(cfg.heads_per_core):
        PT[h * DH:(h + 1) * DH, h * DH:(h + 1) * DH] = PT1
    PT = PT.astype(BF16)
    # causal masks for diagonal kb of each q-chunk
    tri = np.zeros((cfg.kb_per_qc, P, cfg.qc_w), np.float32)
    qpos = np.arange(cfg.qc_w)
    for j in range(cfg.kb_per_qc):
        kpos = j * P + np.arange(P)
        tri[j] = (kpos[:, None] <= qpos[None, :]).astype(np.float32)
    tri = tri.astype(BF16)

    scale_q = DH ** -0.5
    # shared (replicated) weight prep, done once
    wq_l, wo_l, w1_l, w2_l, b1_l, b2_l = [], [], [], [], [], []
    for l in range(cfg.depth):
        g = attn_g[l] * sD
        wqkv3 = w_qkv[l].reshape(D, 3, H * DH)
        wq = (g[:, None] * wqkv3[:, 0, :]) * scale_q
        wk = g[:, None] * wqkv3[:, 1, :]
        wv = g[:, None] * wqkv3[:, 2, :]
        wq_l.append(np.stack([wq, wk, wv], 0))
        wo_l.append(w_out[l])
        gf = ff_g[l] * sD
        w1_l.append(gf[:, None] * w_ff1[l])
        w2_l.append(w_ff2[l])
        b1_l.append(b_ff1[l].reshape(cfg.fb, P).T)
        b2_l.append(b_ff2[l].reshape(cfg.db, P).T)
    wqkv_all = np.stack(wq_l, 0).astype(BF16)           # [depth,3,D,D]
    # out-proj: [depth, db, P, NC, P]
    wout_all = (np.stack(wo_l, 0).reshape(cfg.depth, NC, P, cfg.db, P)
                .transpose(0, 3, 2, 1, 4).copy().astype(BF16))
    # ff1: [depth, fb, P, db, P]
    wff1_all = (np.stack(w1_l, 0).reshape(cfg.depth, cfg.db, P, cfg.fb, P)
                .transpose(0, 3, 2, 1, 4).copy().astype(BF16))
    bff1_all = np.stack(b1_l, 0).astype(np.float32)
    wff2_all = np.stack(w2_l, 0).astype(BF16)
    bff2_all = np.stack(b2_l, 0).astype(np.float32)
    wlogg = ((fin_g * sD)[:, None] * w_log).astype(BF16)

    in_maps = []
    for c in range(NC):
        ec = emb[x[c * t:(c + 1) * t]]          # [t, D] f32
        ecT = ec.T.reshape(cfg.db, P, t).transpose(1, 0, 2)
        in_maps.append({
            "embT": np.ascontiguousarray(ecT, dtype=np.float32),
            "wqkv": wqkv_all,
            "wout": wout_all,
            "wff1": wff1_all,
            "bff1": bff1_all,
            "wff2": wff2_all,
            "bff2": bff2_all,
            "wlog": wlogg,
            "cosq": np.ascontiguousarray(cos2[:, c * t:(c + 1) * t]),
            "sinq": np.ascontiguousarray(sin2[:, c * t:(c + 1) * t]),
            "rotPT": PT, "trimask": tri,
        })
    return in_maps


_CACHED = {}


def kernel(**inputs):
    import jax
    from jax.sharding import Mesh, PartitionSpec
    from jax.experimental.shard_map import shard_map
    from concourse.bass2jax import (_bass_exec_p, install_neuronx_cc_hook,
                                    partition_id_tensor)
    cfg = FULL
    in_maps = prep_inputs(inputs, cfg)
    if "nc" not in _CACHED:
        _CACHED["nc"] = build_kernel(cfg)
    nc = _CACHED["nc"]
    install_neuronx_cc_hook()
    partition_name = nc.partition_id_tensor.name if nc.partition_id_tensor else None
    in_names, out_names, out_avals = [], [], []
    for alloc in nc.m.functions[0].allocations:
        if not isinstance(alloc, mybir.MemoryLocationSet):
            continue
        name = alloc.memorylocations[0].name
        if alloc.kind == "ExternalInput":
            if name != partition_name:
                in_names.append(name)
        elif alloc.kind == "ExternalOutput":
            out_names.append(name)
            out_avals.append(jax.core.ShapedArray(
                tuple(alloc.tensor_shape), mybir.dt.np(alloc.dtype)))
    all_in = list(in_names) + list(out_names)
    if partition_name is not None:
        all_in.append(partition_name)
    n_params = len(in_names)

    def _body(*args):
        operands = list(args)
        if partition_name is not None:
            operands.append(partition_id_tensor())
        return tuple(_bass_exec_p.bind(
            *operands, out_avals=tuple(out_avals), in_names=tuple(all_in),
            out_names=tuple(out_names), lowering_input_output_aliases=(),
            sim_require_finite=True, sim_require_nnan=True, nc=nc))

    devices = jax.devices()[:NC]
    mesh = Mesh(np.asarray(devices), ("core",))
    n_outs = len(out_names)
    sharded = jax.jit(
        shard_map(_body, mesh=mesh,
                  in_specs=(PartitionSpec("core"),) * (n_params + n_outs),
                  out_specs=(PartitionSpec("core"),) * n_outs,
                  check_rep=False),
        donate_argnums=tuple(range(n_params, n_params + n_outs)),
        keep_unused=True)
    concat_in = [np.concatenate([np.asarray(in_maps[c][nm]) for c in range(NC)], 0)
                 for nm in in_names]
    zeros = [np.zeros((NC * a.shape[0], *a.shape[1:]), a.dtype) for a in out_avals]
    out = sharded(*concat_in, *zeros)
    logits = np.asarray(out[out_names.index("logits_out")])
    return logits.reshape(B, cfg.n, cfg.v).astype(np.float32)
